# revision 3
# baseline (speedup 1.0000x reference)
"""CrossDomainAttention TRN2 kernel: 8-core data-parallel over batch.

Reference computation (per batch element, a/b are (L, C) slices):
  ap = a.T (C, L);  q = ap@Wq.T+bq; k,v from b.T
  attn = softmax(q @ k.T / sqrt(L)) (C, C)
  out = LN(attn @ v + ap) over L, returned as the raw (C*L) buffer viewed (L, C)

Layouts on-core (per batch element):
  qT, kT: [m, c] = q[c, m]   -> scoresT[d, c] needs no transposes
  v:      [d, m]
  PT[d, c] = exp(scoresT / sqrt(L)); PV accumulated over d in PSUM
  rowsum via DVE accumulation over d-blocks + ones-matmul partition reduce
"""

import numpy as np

B, L, C = 16, 512, 2048
NCORE = 8
NB = B // NCORE          # batch elements per core
P = 128
F = 512                  # matmul free-dim tile
NLC = L // P             # 4  l/m chunks
NDB = C // P             # 16 d-blocks / c-blocks
NCCH = C // F            # 4  c chunks
LN_EPS = 1e-5
INV_SQRT_L = 1.0 / float(np.sqrt(L))

_CACHE = {}


def _build(apply_qkv_bias: bool, apply_gamma_beta: bool):
    import concourse.bass as bass
    import concourse.tile as tile
    from concourse import bacc, mybir
    from concourse.bass import ts, ds
    from concourse.masks import make_identity

    f32 = mybir.dt.float32
    bf16 = mybir.dt.bfloat16
    AF = mybir.ActivationFunctionType
    ALU = mybir.AluOpType

    nc = bacc.Bacc("TRN2", target_bir_lowering=False, debug=False,
                   enable_asserts=False)

    a_d = nc.dram_tensor("a", (NB, L, C), f32, kind="ExternalInput").ap()
    b_d = nc.dram_tensor("b", (NB, L, C), f32, kind="ExternalInput").ap()
    w_d = {n: nc.dram_tensor(n, (L, L), f32, kind="ExternalInput").ap()
           for n in ("Wq", "Wk", "Wv")}
    bias_d = {n: nc.dram_tensor(n, (L,), f32, kind="ExternalInput").ap()
              for n in ("bq", "bk", "bv")}
    gamma_d = nc.dram_tensor("gamma", (L,), f32, kind="ExternalInput").ap()
    beta_d = nc.dram_tensor("beta", (L,), f32, kind="ExternalInput").ap()
    out_d = nc.dram_tensor("out", (NB, C, L), f32, kind="ExternalOutput").ap()

    def bcast_p(ap1d):
        # broadcast a 1-D DRAM AP across all 128 partitions (DMA source)
        return bass.AP(tensor=ap1d.tensor, offset=ap1d.offset,
                       ap=[[0, P]] + [list(d) for d in ap1d.ap])

    from contextlib import ExitStack
    with tile.TileContext(nc) as tc, ExitStack() as ctx:
        const = ctx.enter_context(tc.tile_pool(name="const", bufs=1))
        inp = ctx.enter_context(tc.tile_pool(name="inp", bufs=1))
        acts = ctx.enter_context(tc.tile_pool(name="acts", bufs=1))
        small = ctx.enter_context(tc.tile_pool(name="small", bufs=2))
        outp = ctx.enter_context(tc.tile_pool(name="outp", bufs=2))
        ps_mm = ctx.enter_context(tc.tile_pool(name="ps_mm", bufs=3, space="PSUM"))
        ps_out = ctx.enter_context(tc.tile_pool(name="ps_out", bufs=2, space="PSUM"))
        ps_tr = ctx.enter_context(tc.tile_pool(name="ps_tr", bufs=2, space="PSUM"))
        ps_rs = ctx.enter_context(tc.tile_pool(name="ps_rs", bufs=1, space="PSUM"))

        # ---- constants ----
        ident = const.tile([P, P], f32, tag="ident")
        make_identity(nc, ident)
        cpack = const.tile([P, 16], f32, tag="cpack")
        nc.vector.memset(cpack[:, 0:1], 1.0)      # ones column (reduce rhs)
        nc.vector.memset(cpack[:, 1:2], LN_EPS)
        ones = cpack[:, 0:1]
        eps = cpack[:, 1:2]
        bias_col = {}
        bv_bc = None
        if apply_qkv_bias:
            for i, n in enumerate(("bq", "bk")):
                dst = cpack[:, 2 + 4 * i: 2 + 4 * (i + 1)]
                nc.sync.dma_start(dst, bias_d[n].rearrange("(o p) -> p o", p=P))
                bias_col[n] = dst
            bv_bc = const.tile([P, L], f32, tag="bv_bc")
            nc.sync.dma_start(bv_bc[:], bcast_p(bias_d["bv"]))
        if apply_gamma_beta:
            gamma_bc = const.tile([P, L], f32, tag="gamma")
            beta_bc = const.tile([P, L], f32, tag="beta")
            nc.sync.dma_start(gamma_bc[:], bcast_p(gamma_d))
            nc.sync.dma_start(beta_bc[:], bcast_p(beta_d))

        # ---- weights: load W[m, l], transpose to WT[l_p, li, m] ----
        WT = {}
        for n in ("Wq", "Wk", "Wv"):
            wld = inp.tile([P, NLC, F], f32, tag="a")  # reuse 'a' slot pre-batch0
            nc.sync.dma_start(wld[:], w_d[n].rearrange("(o p) l -> p o l", p=P))
            wt = const.tile([P, NLC, L], f32, tag=f"WT_{n}")
            for mi in range(NLC):
                for li in range(NLC):
                    pst = ps_tr.tile([P, P], f32, tag="tr")
                    nc.tensor.transpose(pst[:], wld[:, mi, ts(li, P)], ident[:])
                    nc.vector.tensor_copy(wt[:, li, ts(mi, P)], pst[:])
            WT[n] = wt

        # ---- per batch element ----
        for bi in range(NB):
            a_sb = inp.tile([P, NLC, C], f32, tag="a")
            nc.sync.dma_start(a_sb[:], a_d[bi].rearrange("(o p) c -> p o c", p=P))
            b_sb = inp.tile([P, NLC, C], f32, tag="b")
            nc.sync.dma_start(b_sb[:], b_d[bi].rearrange("(o p) c -> p o c", p=P))

            # qT[m_p, mi, c], kT[m_p, mi, c] (bf16)
            qT = acts.tile([P, NLC, C], bf16, tag="qT")
            kT = acts.tile([P, NLC, C], bf16, tag="kT")
            for wname, bname, src, dst in (("Wq", "bq", a_sb, qT),
                                           ("Wk", "bk", b_sb, kT)):
                for mi in range(NLC):
                    for ci in range(NCCH):
                        ps = ps_mm.tile([P, F], f32, tag="mm")
                        for li in range(NLC):
                            nc.tensor.matmul(ps[:],
                                             lhsT=WT[wname][:, li, ts(mi, P)],
                                             rhs=src[:, li, ts(ci, F)],
                                             start=(li == 0), stop=(li == NLC - 1))
                        dslice = dst[:, mi, ts(ci, F)]
                        if apply_qkv_bias:
                            nc.scalar.activation(dslice, ps[:], AF.Identity,
                                                 bias=bias_col[bname][:, mi:mi + 1])
                        else:
                            nc.scalar.copy(dslice, ps[:])

            # v[d_p, di, m] (bf16)
            v_sb = acts.tile([P, NDB, L], bf16, tag="v")
            for di in range(NDB):
                ps = ps_mm.tile([P, F], f32, tag="mm")
                for li in range(NLC):
                    nc.tensor.matmul(ps[:], lhsT=b_sb[:, li, ts(di, P)],
                                     rhs=WT["Wv"][:, li, :],
                                     start=(li == 0), stop=(li == NLC - 1))
                if apply_qkv_bias:
                    # bv is along the free dim: broadcast add via bias columns
                    # (bv[m] identical for all partitions) -> use activation w/
                    # per-partition bias is wrong here; do copy + vector add.
                    nc.scalar.copy(v_sb[:, di, :], ps[:])
                    nc.vector.tensor_add(v_sb[:, di, :], v_sb[:, di, :],
                                         bv_bc[:, :])
                else:
                    nc.scalar.copy(v_sb[:, di, :], ps[:])

            # ---- attention, c-chunk at a time ----
            for ci in range(NCCH):
                PT = acts.tile([P, NDB, F], bf16, tag="PT")
                sumacc = small.tile([P, F], f32, tag="sumacc")
                for di in range(NDB):
                    ps = ps_mm.tile([P, F], f32, tag="mm")
                    for mi in range(NLC):
                        nc.tensor.matmul(ps[:], lhsT=kT[:, mi, ts(di, P)],
                                         rhs=qT[:, mi, ts(ci, F)],
                                         start=(mi == 0), stop=(mi == NLC - 1))
                    nc.scalar.activation(PT[:, di, :], ps[:], AF.Exp,
                                         scale=INV_SQRT_L)
                    if di == 0:
                        nc.vector.tensor_copy(sumacc[:], PT[:, di, :])
                    else:
                        nc.vector.tensor_add(sumacc[:], sumacc[:], PT[:, di, :])

                for cb in range(NCCH):
                    gb = ci * NCCH + cb
                    # rowsum over partitions for these 128 queries
                    psr = ps_rs.tile([P, 1], f32, tag="rs")
                    nc.tensor.matmul(psr[:], lhsT=sumacc[:, ts(cb, P)],
                                     rhs=ones, start=True, stop=True)
                    stats = small.tile([P, 16], f32, tag="stats")
                    rs = stats[:, 0:1]
                    nc.vector.reciprocal(rs, psr[:])
                    # PV
                    po = ps_out.tile([P, L], f32, tag="out")
                    for di in range(NDB):
                        nc.tensor.matmul(po[:], lhsT=PT[:, di, ts(cb, P)],
                                         rhs=v_sb[:, di, :],
                                         start=(di == 0), stop=(di == NDB - 1))
                    out_sb = outp.tile([P, L], f32, tag="out")
                    nc.vector.tensor_scalar_mul(out_sb[:], po[:], rs)
                    # + residual ap[c, l] = a[l, c] via PE transpose
                    for li in range(NLC):
                        pst = ps_tr.tile([P, P], f32, tag="tr")
                        nc.tensor.transpose(pst[:], a_sb[:, li, ts(gb, P)],
                                            ident[:])
                        nc.vector.tensor_add(out_sb[:, ts(li, P)],
                                             out_sb[:, ts(li, P)], pst[:])
                    # LayerNorm over free dim (L)
                    st6 = stats[:, 2:8]
                    mv = stats[:, 8:10]
                    rstd = stats[:, 10:11]
                    nc.vector.bn_stats(st6, out_sb[:])
                    nc.vector.bn_aggr(mv, st6)
                    nc.scalar.activation(rstd, mv[:, 1:2], AF.Sqrt, bias=eps)
                    nc.vector.reciprocal(rstd, rstd)
                    nc.vector.tensor_scalar(out_sb[:], out_sb[:],
                                            mv[:, 0:1], rstd,
                                            ALU.subtract, ALU.mult)
                    if apply_gamma_beta:
                        nc.vector.tensor_mul(out_sb[:], out_sb[:], gamma_bc[:])
                        nc.vector.tensor_add(out_sb[:], out_sb[:], beta_bc[:])
                    nc.sync.dma_start(out_d[bi, ds(gb * P, P), :], out_sb[:])

    nc.compile()
    return nc


def _get_nc(apply_qkv_bias, apply_gamma_beta):
    key = (apply_qkv_bias, apply_gamma_beta)
    if key not in _CACHE:
        _CACHE[key] = _build(*key)
    return _CACHE[key]


def _run(inputs, trace=False):
    from concourse import bass_utils

    a = np.ascontiguousarray(np.asarray(inputs["a"], dtype=np.float32))
    b = np.ascontiguousarray(np.asarray(inputs["b"], dtype=np.float32))
    get = lambda n: np.ascontiguousarray(np.asarray(inputs[n], dtype=np.float32))
    Wq, Wk, Wv = get("Wq"), get("Wk"), get("Wv")
    bq, bk, bv = get("bq"), get("bk"), get("bv")
    gamma, beta = get("gamma"), get("beta")

    apply_qkv_bias = bool(np.any(bq) or np.any(bk) or np.any(bv))
    apply_gamma_beta = bool(np.any(gamma != 1.0) or np.any(beta))
    nc = _get_nc(apply_qkv_bias, apply_gamma_beta)

    in_maps = []
    for c in range(NCORE):
        sl = slice(c * NB, (c + 1) * NB)
        in_maps.append({
            "a": np.ascontiguousarray(a[sl]), "b": np.ascontiguousarray(b[sl]),
            "Wq": Wq, "Wk": Wk, "Wv": Wv,
            "bq": bq, "bk": bk, "bv": bv,
            "gamma": gamma, "beta": beta,
        })
    res = bass_utils.run_bass_kernel_spmd(nc, in_maps,
                                          core_ids=list(range(NCORE)),
                                          trace=trace)
    out = np.concatenate(
        [res.results[c]["out"].reshape(NB, L, C) for c in range(NCORE)], axis=0)
    return out, res


def kernel(**inputs):
    out, _ = _run(inputs, trace=False)
    return out


# revision 11
# speedup vs baseline: 1.0313x; 1.0313x over previous
"""CrossDomainAttention TRN2 kernel: 8-core data-parallel over batch.

Reference computation (per batch element, a/b are (L, C) slices):
  ap = a.T (C, L);  q = ap@Wq.T+bq; k,v from b.T
  attn = softmax(q @ k.T / sqrt(L)) (C, C)
  out = LN(attn @ v + ap) over L, returned as the raw (C*L) buffer viewed (L, C)

Layouts on-core (per batch element):
  qT, kT: [m, c] = q[c, m]   -> scoresT[d, c] needs no transposes
  v:      [d, m]
  PT[d, c] = exp(scoresT / sqrt(L)); PV accumulated over d in PSUM
  rowsum via DVE accumulation over d-blocks + ones-matmul partition reduce
  residual ap pre-transposed (PE) into fp16 apT; all big matmuls bf16.
"""

import numpy as np

B, L, C = 16, 512, 2048
NCORE = 8
NB = B // NCORE          # batch elements per core
P = 128
F = 512                  # matmul free-dim tile
NLC = L // P             # 4  l/m chunks
NDB = C // P             # 16 d-blocks / c-blocks
NCCH = C // F            # 4  c chunks
LN_EPS = 1e-5
INV_SQRT_L = 1.0 / float(np.sqrt(L))

_CACHE = {}


def _build(apply_qkv_bias: bool, apply_gamma_beta: bool):
    import concourse.bass as bass
    import concourse.tile as tile
    from concourse import bacc, mybir
    from concourse.bass import ts, ds
    from concourse.masks import make_identity
    from contextlib import ExitStack

    f32 = mybir.dt.float32
    f16 = mybir.dt.float16
    bf16 = mybir.dt.bfloat16
    AF = mybir.ActivationFunctionType
    ALU = mybir.AluOpType

    nc = bacc.Bacc("TRN2", target_bir_lowering=False, debug=False,
                   enable_asserts=False)

    a_d = nc.dram_tensor("a", (NB, L, C), f32, kind="ExternalInput").ap()
    b_d = nc.dram_tensor("b", (NB, L, C), f32, kind="ExternalInput").ap()
    w_d = {n: nc.dram_tensor(n, (L, L), f32, kind="ExternalInput").ap()
           for n in ("Wq", "Wk", "Wv")}
    bias_d = {n: nc.dram_tensor(n, (L,), f32, kind="ExternalInput").ap()
              for n in ("bq", "bk", "bv")}
    gamma_d = nc.dram_tensor("gamma", (L,), f32, kind="ExternalInput").ap()
    beta_d = nc.dram_tensor("beta", (L,), f32, kind="ExternalInput").ap()
    out_d = nc.dram_tensor("out", (NB, C, L), f32, kind="ExternalOutput").ap()

    def bcast_p(ap1d):
        # broadcast a 1-D DRAM AP across all 128 partitions (DMA source)
        return bass.AP(tensor=ap1d.tensor, offset=ap1d.offset,
                       ap=[[0, P]] + [list(d) for d in ap1d.ap])

    with tile.TileContext(nc) as tc, ExitStack() as ctx:
        const = ctx.enter_context(tc.tile_pool(name="const", bufs=1))
        inp = ctx.enter_context(tc.tile_pool(name="inp", bufs=1))
        acts = ctx.enter_context(tc.tile_pool(name="acts", bufs=1))
        small = ctx.enter_context(tc.tile_pool(name="small", bufs=2))
        outp = ctx.enter_context(tc.tile_pool(name="outp", bufs=2))
        ps_mm = ctx.enter_context(tc.tile_pool(name="ps_mm", bufs=4, space="PSUM"))
        ps_out = ctx.enter_context(tc.tile_pool(name="ps_out", bufs=2, space="PSUM"))
        ps_tr = ctx.enter_context(tc.tile_pool(name="ps_tr", bufs=1, space="PSUM"))
        ps_rs = ctx.enter_context(tc.tile_pool(name="ps_rs", bufs=1, space="PSUM"))

        # ---- constants ----
        ident = const.tile([P, P], f32, tag="ident")
        make_identity(nc, ident)
        cpack = const.tile([P, 16], f32, tag="cpack")
        nc.vector.memset(cpack[:, 0:1], 1.0)      # ones column (reduce rhs)
        nc.vector.memset(cpack[:, 1:2], LN_EPS)
        ones = cpack[:, 0:1]
        eps = cpack[:, 1:2]
        bias_col = {}
        bv_bc = None
        if apply_qkv_bias:
            for i, n in enumerate(("bq", "bk")):
                dst = cpack[:, 2 + 4 * i: 2 + 4 * (i + 1)]
                nc.sync.dma_start(dst, bias_d[n].rearrange("(o p) -> p o", p=P))
                bias_col[n] = dst
            bv_bc = const.tile([P, L], f32, tag="bv_bc")
            nc.sync.dma_start(bv_bc[:], bcast_p(bias_d["bv"]))
        if apply_gamma_beta:
            gb_pack = const.tile([P, 2, L], f32, tag="gb")
            nc.sync.dma_start(gb_pack[:, 0, :], bcast_p(gamma_d))
            nc.sync.dma_start(gb_pack[:, 1, :], bcast_p(beta_d))

        # ---- weights: load W[m, l] fp32, transpose -> WT[l_p, li, m] bf16 ----
        WT = {}
        for n in ("Wq", "Wk", "Wv"):
            wld = inp.tile([P, NLC, F], f32, tag="bh")
            nc.sync.dma_start(wld[:], w_d[n].rearrange("(o p) l -> p o l", p=P))
            wt = const.tile([P, NLC, L], bf16, tag=f"WT_{n}")
            for mi in range(NLC):
                pst = ps_tr.tile([P, F], f32, tag="tr")
                for li in range(NLC):
                    nc.tensor.transpose(pst[:, ts(li, P)],
                                        wld[:, mi, ts(li, P)], ident[:])
                nc.vector.tensor_copy(
                    wt[:, :, ts(mi, P)],
                    pst.rearrange("p (li f) -> p li f", f=P))
            WT[n] = wt

        # ---- per batch element ----
        for bi in range(NB):
            # a: load fp32 per l-chunk, cast to bf16 (GPSIMD)
            a_sb = inp.tile([P, NLC, C], f32, tag="a")
            a_bf = acts.tile([P, NLC, C], bf16, tag="a_bf")
            for li in range(NLC):
                nc.sync.dma_start(a_sb[:, li, :],
                                  a_d[bi, ds(li * P, P), :])
                nc.gpsimd.tensor_copy(a_bf[:, li, :], a_sb[:, li, :])
            apT = acts.tile([P, NDB, L], f16, tag="apT")
            for gb in range(NDB):
                pst = ps_tr.tile([P, F], f32, tag="tr")
                for li in range(NLC):
                    nc.tensor.transpose(pst[:, ts(li, P)],
                                        a_sb[:, li, ts(gb, P)], ident[:])
                nc.vector.tensor_copy(apT[:, gb, :], pst[:])

            # b: load fp32 in halves, cast to bf16
            b_bf = acts.tile([P, NLC, C], bf16, tag="bpt", bufs=2)
            for h in range(2):
                b_sb = inp.tile([P, 2, C], f32, tag="bh")
                nc.sync.dma_start(
                    b_sb[:],
                    b_d[bi, ds(h * 2 * P, 2 * P), :].rearrange(
                        "(o p) c -> p o c", p=P))
                for li in range(2):
                    nc.gpsimd.tensor_copy(b_bf[:, h * 2 + li, :], b_sb[:, li, :])

            # qT[m_p, mi, c], kT[m_p, mi, c] (bf16)
            qT = acts.tile([P, NLC, C], bf16, tag="qT")
            kT = acts.tile([P, NLC, C], bf16, tag="kT")
            for wname, bname, src, dst in (("Wq", "bq", a_bf, qT),
                                           ("Wk", "bk", b_bf, kT)):
                for mi in range(NLC):
                    for ci in range(NCCH):
                        ps = ps_mm.tile([P, F], f32, tag="mm")
                        for li in range(NLC):
                            nc.tensor.matmul(ps[:],
                                             lhsT=WT[wname][:, li, ts(mi, P)],
                                             rhs=src[:, li, ts(ci, F)],
                                             start=(li == 0), stop=(li == NLC - 1))
                        dslice = dst[:, mi, ts(ci, F)]
                        if apply_qkv_bias:
                            nc.scalar.activation(dslice, ps[:], AF.Identity,
                                                 bias=bias_col[bname][:, mi:mi + 1])
                        elif (mi + ci) % 2 == 0:
                            nc.scalar.copy(dslice, ps[:])
                        else:
                            nc.vector.tensor_copy(dslice, ps[:])

            # v[d_p, di, m] (bf16)
            v_sb = acts.tile([P, NDB, L], bf16, tag="v")
            for di in range(NDB):
                ps = ps_mm.tile([P, F], f32, tag="mm")
                for li in range(NLC):
                    nc.tensor.matmul(ps[:], lhsT=b_bf[:, li, ts(di, P)],
                                     rhs=WT["Wv"][:, li, :],
                                     start=(li == 0), stop=(li == NLC - 1))
                nc.scalar.copy(v_sb[:, di, :], ps[:])
                if apply_qkv_bias:
                    nc.vector.tensor_add(v_sb[:, di, :], v_sb[:, di, :],
                                         bv_bc[:, :])

            # ---- attention, c-chunk at a time ----
            for ci in range(NCCH):
                PT = acts.tile([P, NDB, F], bf16, tag="bpt", bufs=2)
                sumacc = small.tile([P, F], f32, tag="sumacc")
                for di in range(NDB):
                    ps = ps_mm.tile([P, F], f32, tag="mm")
                    for mi in range(NLC):
                        nc.tensor.matmul(ps[:], lhsT=kT[:, mi, ts(di, P)],
                                         rhs=qT[:, mi, ts(ci, F)],
                                         start=(mi == 0), stop=(mi == NLC - 1))
                    nc.scalar.activation(PT[:, di, :], ps[:], AF.Exp,
                                         scale=INV_SQRT_L)
                    if di == 0:
                        nc.vector.tensor_copy(sumacc[:], PT[:, di, :])
                    else:
                        nc.vector.tensor_add(sumacc[:], sumacc[:], PT[:, di, :])

                for cb in range(NCCH):
                    gb = ci * NCCH + cb
                    # rowsum over partitions for these 128 queries
                    psr = ps_rs.tile([P, 1], f32, tag="rs")
                    nc.tensor.matmul(psr[:], lhsT=sumacc[:, ts(cb, P)],
                                     rhs=ones, start=True, stop=True)
                    stats = small.tile([P, 16], f32, tag="stats")
                    rs = stats[:, 0:1]
                    nc.vector.reciprocal(rs, psr[:])
                    # PV
                    po = ps_out.tile([P, L], f32, tag="out")
                    for di in range(NDB):
                        nc.tensor.matmul(po[:], lhsT=PT[:, di, ts(cb, P)],
                                         rhs=v_sb[:, di, :],
                                         start=(di == 0), stop=(di == NDB - 1))
                    out_sb = outp.tile([P, L], f32, tag="out")
                    nc.vector.tensor_scalar_mul(out_sb[:], po[:], rs)
                    nc.vector.tensor_add(out_sb[:], out_sb[:], apT[:, gb, :])
                    # LayerNorm over free dim (L)
                    st6 = stats[:, 2:8]
                    mv = stats[:, 8:10]
                    rstd = stats[:, 10:11]
                    nc.vector.bn_stats(st6, out_sb[:])
                    nc.vector.bn_aggr(mv, st6)
                    nc.scalar.activation(rstd, mv[:, 1:2], AF.Sqrt, bias=eps)
                    nc.vector.reciprocal(rstd, rstd)
                    nc.vector.tensor_scalar(out_sb[:], out_sb[:],
                                            mv[:, 0:1], rstd,
                                            ALU.subtract, ALU.mult)
                    if apply_gamma_beta:
                        nc.vector.tensor_mul(out_sb[:], out_sb[:],
                                             gb_pack[:, 0, :])
                        nc.vector.tensor_add(out_sb[:], out_sb[:],
                                             gb_pack[:, 1, :])
                    nc.sync.dma_start(out_d[bi, ds(gb * P, P), :], out_sb[:])

    nc.compile()
    return nc


def _get_nc(apply_qkv_bias, apply_gamma_beta):
    key = (apply_qkv_bias, apply_gamma_beta)
    if key not in _CACHE:
        _CACHE[key] = _build(*key)
    return _CACHE[key]


def _run(inputs, trace=False):
    from concourse import bass_utils

    a = np.ascontiguousarray(np.asarray(inputs["a"], dtype=np.float32))
    b = np.ascontiguousarray(np.asarray(inputs["b"], dtype=np.float32))
    get = lambda n: np.ascontiguousarray(np.asarray(inputs[n], dtype=np.float32))
    Wq, Wk, Wv = get("Wq"), get("Wk"), get("Wv")
    bq, bk, bv = get("bq"), get("bk"), get("bv")
    gamma, beta = get("gamma"), get("beta")

    apply_qkv_bias = bool(np.any(bq) or np.any(bk) or np.any(bv))
    apply_gamma_beta = bool(np.any(gamma != 1.0) or np.any(beta))
    nc = _get_nc(apply_qkv_bias, apply_gamma_beta)

    in_maps = []
    for c in range(NCORE):
        sl = slice(c * NB, (c + 1) * NB)
        in_maps.append({
            "a": np.ascontiguousarray(a[sl]), "b": np.ascontiguousarray(b[sl]),
            "Wq": Wq, "Wk": Wk, "Wv": Wv,
            "bq": bq, "bk": bk, "bv": bv,
            "gamma": gamma, "beta": beta,
        })
    res = bass_utils.run_bass_kernel_spmd(nc, in_maps,
                                          core_ids=list(range(NCORE)),
                                          trace=trace)
    out = np.concatenate(
        [res.results[c]["out"].reshape(NB, L, C) for c in range(NCORE)], axis=0)
    return out, res


def kernel(**inputs):
    out, _ = _run(inputs, trace=False)
    return out


# revision 12
# speedup vs baseline: 222.3449x; 215.6004x over previous
"""CrossDomainAttention TRN2 kernel: 8-core data-parallel over batch.

Reference computation (per batch element, a/b are (L, C) slices):
  ap = a.T (C, L);  q = ap@Wq.T+bq; k,v from b.T
  attn = softmax(q @ k.T / sqrt(L)) (C, C)
  out = LN(attn @ v + ap) over L, returned as the raw (C*L) buffer viewed (L, C)

Layouts on-core (per batch element):
  qT, kT: [m, c] = q[c, m]   -> scoresT[d, c] needs no transposes
  v:      [d, m]
  PT[d, c] = exp(scoresT / sqrt(L)); PV accumulated over d in PSUM
  rowsum via DVE accumulation over d-blocks + ones-matmul partition reduce
  residual ap pre-transposed (PE) into fp16 apT; all big matmuls bf16.
"""

import numpy as np

B, L, C = 16, 512, 2048
NCORE = 8
NB = B // NCORE          # batch elements per core
P = 128
F = 512                  # matmul free-dim tile
NLC = L // P             # 4  l/m chunks
NDB = C // P             # 16 d-blocks / c-blocks
NCCH = C // F            # 4  c chunks
LN_EPS = 1e-5
INV_SQRT_L = 1.0 / float(np.sqrt(L))

_CACHE = {}


def _build(apply_qkv_bias: bool, apply_gamma_beta: bool, repeat: int = 1):
    import concourse.bass as bass
    import concourse.tile as tile
    from concourse import bacc, mybir
    from concourse.bass import ts, ds
    from concourse.masks import make_identity
    from contextlib import ExitStack

    f32 = mybir.dt.float32
    f16 = mybir.dt.float16
    bf16 = mybir.dt.bfloat16
    AF = mybir.ActivationFunctionType
    ALU = mybir.AluOpType

    nc = bacc.Bacc("TRN2", target_bir_lowering=False, debug=False,
                   enable_asserts=False)

    a_d = nc.dram_tensor("a", (NB, L, C), f32, kind="ExternalInput").ap()
    b_d = nc.dram_tensor("b", (NB, L, C), f32, kind="ExternalInput").ap()
    w_d = {n: nc.dram_tensor(n, (L, L), f32, kind="ExternalInput").ap()
           for n in ("Wq", "Wk", "Wv")}
    bias_d = {n: nc.dram_tensor(n, (L,), f32, kind="ExternalInput").ap()
              for n in ("bq", "bk", "bv")}
    gamma_d = nc.dram_tensor("gamma", (L,), f32, kind="ExternalInput").ap()
    beta_d = nc.dram_tensor("beta", (L,), f32, kind="ExternalInput").ap()
    out_d = nc.dram_tensor("out", (NB, C, L), f32, kind="ExternalOutput").ap()

    def bcast_p(ap1d):
        # broadcast a 1-D DRAM AP across all 128 partitions (DMA source)
        return bass.AP(tensor=ap1d.tensor, offset=ap1d.offset,
                       ap=[[0, P]] + [list(d) for d in ap1d.ap])

    with tile.TileContext(nc) as tc, ExitStack() as ctx:
        const = ctx.enter_context(tc.tile_pool(name="const", bufs=1))
        inp = ctx.enter_context(tc.tile_pool(name="inp", bufs=1))
        acts = ctx.enter_context(tc.tile_pool(name="acts", bufs=1))
        small = ctx.enter_context(tc.tile_pool(name="small", bufs=2))
        outp = ctx.enter_context(tc.tile_pool(name="outp", bufs=2))
        ps_mm = ctx.enter_context(tc.tile_pool(name="ps_mm", bufs=4, space="PSUM"))
        ps_out = ctx.enter_context(tc.tile_pool(name="ps_out", bufs=2, space="PSUM"))
        ps_tr = ctx.enter_context(tc.tile_pool(name="ps_tr", bufs=1, space="PSUM"))
        ps_rs = ctx.enter_context(tc.tile_pool(name="ps_rs", bufs=1, space="PSUM"))

        # ---- constants ----
        ident = const.tile([P, P], f32, tag="ident")
        make_identity(nc, ident)
        cpack = const.tile([P, 16], f32, tag="cpack")
        nc.vector.memset(cpack[:, 0:1], 1.0)      # ones column (reduce rhs)
        nc.vector.memset(cpack[:, 1:2], LN_EPS)
        ones = cpack[:, 0:1]
        eps = cpack[:, 1:2]
        bias_col = {}
        bv_bc = None
        if apply_qkv_bias:
            for i, n in enumerate(("bq", "bk")):
                dst = cpack[:, 2 + 4 * i: 2 + 4 * (i + 1)]
                nc.sync.dma_start(dst, bias_d[n].rearrange("(o p) -> p o", p=P))
                bias_col[n] = dst
            bv_bc = const.tile([P, L], f32, tag="bv_bc")
            nc.sync.dma_start(bv_bc[:], bcast_p(bias_d["bv"]))
        if apply_gamma_beta:
            gb_pack = const.tile([P, 2, L], f32, tag="gb")
            nc.sync.dma_start(gb_pack[:, 0, :], bcast_p(gamma_d))
            nc.sync.dma_start(gb_pack[:, 1, :], bcast_p(beta_d))

        # ---- weights: load W[m, l] fp32, transpose -> WT[l_p, li, m] bf16 ----
        WT = {}
        for n in ("Wq", "Wk", "Wv"):
            wld = inp.tile([P, NLC, F], f32, tag="bh")
            nc.sync.dma_start(wld[:], w_d[n].rearrange("(o p) l -> p o l", p=P))
            wt = const.tile([P, NLC, L], bf16, tag=f"WT_{n}")
            for mi in range(NLC):
                pst = ps_tr.tile([P, F], f32, tag="tr")
                for li in range(NLC):
                    nc.tensor.transpose(pst[:, ts(li, P)],
                                        wld[:, mi, ts(li, P)], ident[:])
                nc.vector.tensor_copy(
                    wt[:, :, ts(mi, P)],
                    pst.rearrange("p (li f) -> p li f", f=P))
            WT[n] = wt

        # ---- per batch element ----
        for bi in [i % NB for i in range(NB * repeat)]:
            # a: load fp32 per l-chunk, cast to bf16 (GPSIMD)
            a_sb = inp.tile([P, NLC, C], f32, tag="a")
            a_bf = acts.tile([P, NLC, C], bf16, tag="a_bf")
            for li in range(NLC):
                nc.sync.dma_start(a_sb[:, li, :],
                                  a_d[bi, ds(li * P, P), :])
                nc.gpsimd.tensor_copy(a_bf[:, li, :], a_sb[:, li, :])
            apT = acts.tile([P, NDB, L], f16, tag="apT")
            for gb in range(NDB):
                pst = ps_tr.tile([P, F], f32, tag="tr")
                for li in range(NLC):
                    nc.tensor.transpose(pst[:, ts(li, P)],
                                        a_sb[:, li, ts(gb, P)], ident[:])
                nc.vector.tensor_copy(apT[:, gb, :], pst[:])

            # b: load fp32 in halves, cast to bf16
            b_bf = acts.tile([P, NLC, C], bf16, tag="bpt", bufs=2)
            for h in range(2):
                b_sb = inp.tile([P, 2, C], f32, tag="bh")
                nc.sync.dma_start(
                    b_sb[:],
                    b_d[bi, ds(h * 2 * P, 2 * P), :].rearrange(
                        "(o p) c -> p o c", p=P))
                for li in range(2):
                    nc.gpsimd.tensor_copy(b_bf[:, h * 2 + li, :], b_sb[:, li, :])

            # qT[m_p, mi, c], kT[m_p, mi, c] (bf16)
            qT = acts.tile([P, NLC, C], bf16, tag="qT")
            kT = acts.tile([P, NLC, C], bf16, tag="kT")
            for wname, bname, src, dst in (("Wq", "bq", a_bf, qT),
                                           ("Wk", "bk", b_bf, kT)):
                for mi in range(NLC):
                    for ci in range(NCCH):
                        ps = ps_mm.tile([P, F], f32, tag="mm")
                        for li in range(NLC):
                            nc.tensor.matmul(ps[:],
                                             lhsT=WT[wname][:, li, ts(mi, P)],
                                             rhs=src[:, li, ts(ci, F)],
                                             start=(li == 0), stop=(li == NLC - 1))
                        dslice = dst[:, mi, ts(ci, F)]
                        if apply_qkv_bias:
                            nc.scalar.activation(dslice, ps[:], AF.Identity,
                                                 bias=bias_col[bname][:, mi:mi + 1])
                        elif (mi + ci) % 2 == 0:
                            nc.scalar.copy(dslice, ps[:])
                        else:
                            nc.vector.tensor_copy(dslice, ps[:])

            # v[d_p, di, m] (bf16)
            v_sb = acts.tile([P, NDB, L], bf16, tag="v")
            for di in range(NDB):
                ps = ps_mm.tile([P, F], f32, tag="mm")
                for li in range(NLC):
                    nc.tensor.matmul(ps[:], lhsT=b_bf[:, li, ts(di, P)],
                                     rhs=WT["Wv"][:, li, :],
                                     start=(li == 0), stop=(li == NLC - 1))
                nc.scalar.copy(v_sb[:, di, :], ps[:])
                if apply_qkv_bias:
                    nc.vector.tensor_add(v_sb[:, di, :], v_sb[:, di, :],
                                         bv_bc[:, :])

            # ---- attention, c-chunk at a time ----
            for ci in range(NCCH):
                PT = acts.tile([P, NDB, F], bf16, tag="bpt", bufs=2)
                sumacc = small.tile([P, F], f32, tag="sumacc")
                for di in range(NDB):
                    ps = ps_mm.tile([P, F], f32, tag="mm")
                    for mi in range(NLC):
                        nc.tensor.matmul(ps[:], lhsT=kT[:, mi, ts(di, P)],
                                         rhs=qT[:, mi, ts(ci, F)],
                                         start=(mi == 0), stop=(mi == NLC - 1))
                    nc.scalar.activation(PT[:, di, :], ps[:], AF.Exp,
                                         scale=INV_SQRT_L)
                    if di == 0:
                        nc.vector.tensor_copy(sumacc[:], PT[:, di, :])
                    else:
                        nc.vector.tensor_add(sumacc[:], sumacc[:], PT[:, di, :])

                for cb in range(NCCH):
                    gb = ci * NCCH + cb
                    # rowsum over partitions for these 128 queries
                    psr = ps_rs.tile([P, 1], f32, tag="rs")
                    nc.tensor.matmul(psr[:], lhsT=sumacc[:, ts(cb, P)],
                                     rhs=ones, start=True, stop=True)
                    stats = small.tile([P, 16], f32, tag="stats")
                    rs = stats[:, 0:1]
                    nc.vector.reciprocal(rs, psr[:])
                    # PV
                    po = ps_out.tile([P, L], f32, tag="out")
                    for di in range(NDB):
                        nc.tensor.matmul(po[:], lhsT=PT[:, di, ts(cb, P)],
                                         rhs=v_sb[:, di, :],
                                         start=(di == 0), stop=(di == NDB - 1))
                    out_sb = outp.tile([P, L], f32, tag="out")
                    nc.vector.tensor_scalar_mul(out_sb[:], po[:], rs)
                    nc.vector.tensor_add(out_sb[:], out_sb[:], apT[:, gb, :])
                    # LayerNorm over free dim (L)
                    st6 = stats[:, 2:8]
                    mv = stats[:, 8:10]
                    rstd = stats[:, 10:11]
                    nc.vector.bn_stats(st6, out_sb[:])
                    nc.vector.bn_aggr(mv, st6)
                    nc.scalar.activation(rstd, mv[:, 1:2], AF.Sqrt, bias=eps)
                    nc.vector.reciprocal(rstd, rstd)
                    nc.vector.tensor_scalar(out_sb[:], out_sb[:],
                                            mv[:, 0:1], rstd,
                                            ALU.subtract, ALU.mult)
                    if apply_gamma_beta:
                        nc.vector.tensor_mul(out_sb[:], out_sb[:],
                                             gb_pack[:, 0, :])
                        nc.vector.tensor_add(out_sb[:], out_sb[:],
                                             gb_pack[:, 1, :])
                    nc.sync.dma_start(out_d[bi, ds(gb * P, P), :], out_sb[:])

    nc.compile()
    return nc


def _get_nc(apply_qkv_bias, apply_gamma_beta, repeat=1):
    key = (apply_qkv_bias, apply_gamma_beta, repeat)
    if key not in _CACHE:
        _CACHE[key] = _build(*key)
    return _CACHE[key]


def _run(inputs, trace=False):
    from concourse import bass_utils

    a = np.ascontiguousarray(np.asarray(inputs["a"], dtype=np.float32))
    b = np.ascontiguousarray(np.asarray(inputs["b"], dtype=np.float32))
    get = lambda n: np.ascontiguousarray(np.asarray(inputs[n], dtype=np.float32))
    Wq, Wk, Wv = get("Wq"), get("Wk"), get("Wv")
    bq, bk, bv = get("bq"), get("bk"), get("bv")
    gamma, beta = get("gamma"), get("beta")

    apply_qkv_bias = bool(np.any(bq) or np.any(bk) or np.any(bv))
    apply_gamma_beta = bool(np.any(gamma != 1.0) or np.any(beta))
    nc = _get_nc(apply_qkv_bias, apply_gamma_beta)

    in_maps = []
    for c in range(NCORE):
        sl = slice(c * NB, (c + 1) * NB)
        in_maps.append({
            "a": np.ascontiguousarray(a[sl]), "b": np.ascontiguousarray(b[sl]),
            "Wq": Wq, "Wk": Wk, "Wv": Wv,
            "bq": bq, "bk": bk, "bv": bv,
            "gamma": gamma, "beta": beta,
        })
    res = bass_utils.run_bass_kernel_spmd(nc, in_maps,
                                          core_ids=list(range(NCORE)),
                                          trace=trace)
    out = np.concatenate(
        [res.results[c]["out"].reshape(NB, L, C) for c in range(NCORE)], axis=0)
    return out, res


def kernel(**inputs):
    out, _ = _run(inputs, trace=False)
    return out
